# revision 1
# baseline (speedup 1.0000x reference)
import sys

sys.path.insert(0, "/opt/trn_rl_repo")
import numpy as np
import concourse.bass as bass
from concourse import bacc
import concourse.mybir as mybir
import concourse.tile as tile
from concourse.bass_utils import run_bass_kernel_spmd

f32 = mybir.dt.float32
X = mybir.AxisListType.X
MUL = mybir.AluOpType.mult
ADD = mybir.AluOpType.add

B, T, N, D = 16, 12, 1024, 128
H, HD = 8, 16
NCORES = 8
SLICES = (B * T) // NCORES  # 24 slices per core
NT = N // 128  # 8 token tiles per slice

_cached = {}


def _build():
    nc = bacc.Bacc()
    x_sh = nc.dram_tensor("x_sh", [SLICES, N, D], f32, kind="ExternalInput")
    w_qkv = nc.dram_tensor("w_qkv", [D, 3 * D], f32, kind="ExternalInput")
    w_out = nc.dram_tensor("w_out", [D, D], f32, kind="ExternalInput")
    b_out = nc.dram_tensor("b_out", [D], f32, kind="ExternalInput")
    iden = nc.dram_tensor("iden", [128, 128], f32, kind="ExternalInput")
    mblk = nc.dram_tensor("mblk", [128, 128], f32, kind="ExternalInput")
    msel = nc.dram_tensor("msel", [128, H], f32, kind="ExternalInput")
    y_sh = nc.dram_tensor("y_sh", [SLICES, N, D], f32, kind="ExternalOutput")

    with tile.TileContext(nc) as tc:
        with (
            tc.tile_pool(name="consts", bufs=1) as cp,
            tc.tile_pool(name="work", bufs=2) as wp,
            tc.tile_pool(name="qkvs", bufs=10) as qp,
            tc.tile_pool(name="small", bufs=4) as sp,
            tc.tile_pool(name="tp_ps", bufs=2, space="PSUM") as tp,
            tc.tile_pool(name="qkv_ps", bufs=2, space="PSUM") as kp,
            tc.tile_pool(name="g_ps", bufs=1, space="PSUM") as gp,
            tc.tile_pool(name="nd_ps", bufs=2, space="PSUM") as ndp,
            tc.tile_pool(name="fin_ps", bufs=1, space="PSUM") as fp,
        ):
            wq = cp.tile([128, 3 * D], f32)
            nc.sync.dma_start(wq, w_qkv[:, :])
            wo = cp.tile([128, D], f32)
            nc.sync.dma_start(wo, w_out[:, :])
            ident = cp.tile([128, 128], f32)
            nc.sync.dma_start(ident, iden[:, :])
            mb = cp.tile([128, 128], f32)
            nc.sync.dma_start(mb, mblk[:, :])
            ms = cp.tile([128, H], f32)
            nc.sync.dma_start(ms, msel[:, :])
            bias = cp.tile([128, 128], f32)
            bap = b_out[:]
            nc.gpsimd.dma_start(
                out=bias, in_=bass.AP(tensor=bap.tensor, offset=0, ap=[[0, 128], [1, 128]])
            )

            for s in range(SLICES):
                x_in = wp.tile([128, NT, 128], f32, tag="x_in")
                nc.sync.dma_start(
                    x_in, x_sh[s].rearrange("(t p) d -> p t d", p=128)
                )
                xT = wp.tile([128, N], f32, tag="xT")
                qkv_sb = []
                for t in range(NT):
                    pt = tp.tile([128, 128], f32, tag="tp")
                    nc.tensor.transpose(pt, x_in[:, t, :], ident)
                    nc.any.tensor_copy(out=xT[:, t * 128 : (t + 1) * 128], in_=pt)
                for t in range(NT):
                    pk = kp.tile([128, 384], f32, tag="qkv")
                    nc.tensor.matmul(
                        pk, xT[:, t * 128 : (t + 1) * 128], wq, start=True, stop=True
                    )
                    qs = qp.tile([128, 385], f32, tag="qkv_sb")
                    nc.any.tensor_copy(out=qs[:, 0:384], in_=pk)
                    nc.any.memset(qs[:, 384:385], 1.0)
                    qkv_sb.append(qs)
                # normalize q,k per head (16-elem groups)
                recips = []
                for t in range(NT):
                    qs = qkv_sb[t]
                    sq = sp.tile([128, 256], f32, tag="sq")
                    nc.any.tensor_mul(out=sq, in0=qs[:, 0:256], in1=qs[:, 0:256])
                    red = sp.tile([128, 16], f32, tag="red")
                    nc.vector.reduce_sum(
                        out=red, in_=sq.rearrange("p (g e) -> p g e", e=16), axis=X
                    )
                    nrm = sp.tile([128, 16], f32, tag="nrm")
                    nc.scalar.sqrt(nrm, red)
                    nc.any.tensor_scalar_max(nrm, nrm, 1e-12)
                    rcp = sp.tile([128, 16], f32, tag="rcp")
                    nc.vector.reciprocal(rcp, nrm)
                    v16 = qs[:, 0:256].rearrange("p (g e) -> p g e", e=16)
                    nc.any.tensor_mul(
                        out=v16, in0=v16, in1=rcp[:, :, None].to_broadcast((128, 16, 16))
                    )
                    recips.append(rcp)
                # G = ks^T @ [vs | 1]  (accumulate over token tiles)
                g = gp.tile([128, 129], f32, tag="g")
                for t in range(NT):
                    nc.tensor.matmul(
                        g,
                        qkv_sb[t][:, 128:256],
                        qkv_sb[t][:, 256:385],
                        start=(t == 0),
                        stop=(t == NT - 1),
                    )
                gcomb = wp.tile([128, 136], f32, tag="gcomb")
                nc.any.tensor_mul(out=gcomb[:, 0:128], in0=g[:, 0:128], in1=mb)
                nc.any.tensor_scalar_mul(gcomb[:, 128:136], ms, g[:, 128:129])
                # qsT
                qsT = wp.tile([128, N], f32, tag="qsT")
                for t in range(NT):
                    pt = tp.tile([128, 128], f32, tag="tp")
                    nc.tensor.transpose(pt, qkv_sb[t][:, 0:128], ident)
                    nc.any.tensor_copy(out=qsT[:, t * 128 : (t + 1) * 128], in_=pt)
                # nd = qs @ [Gkv | Gks]; then out = (nd_kv + N*vs) / (nd_ks + N)
                resT = wp.tile([128, N], f32, tag="resT")
                for t in range(NT):
                    nd = ndp.tile([128, 136], f32, tag="nd")
                    nc.tensor.matmul(
                        nd, qsT[:, t * 128 : (t + 1) * 128], gcomb, start=True, stop=True
                    )
                    vs1024 = sp.tile([128, 128], f32, tag="vs1024")
                    nc.scalar.mul(out=vs1024, in_=qkv_sb[t][:, 256:384], mul=float(N))
                    num = sp.tile([128, 128], f32, tag="num")
                    nc.any.tensor_add(out=num, in0=nd[:, 0:128], in1=vs1024)
                    den = sp.tile([128, 8], f32, tag="den")
                    nc.any.tensor_scalar_add(den, nd[:, 128:136], float(N))
                    rcd = sp.tile([128, 8], f32, tag="rcd")
                    nc.vector.reciprocal(rcd, den)
                    res = sp.tile([128, 128], f32, tag="res")
                    nc.any.tensor_mul(
                        out=res.rearrange("p (g e) -> p g e", e=16),
                        in0=num.rearrange("p (g e) -> p g e", e=16),
                        in1=rcd[:, :, None].to_broadcast((128, 8, 16)),
                    )
                    pt = tp.tile([128, 128], f32, tag="tp")
                    nc.tensor.transpose(pt, res, ident)
                    nc.any.tensor_copy(out=resT[:, t * 128 : (t + 1) * 128], in_=pt)
                for t in range(NT):
                    pf = fp.tile([128, 128], f32, tag="fin")
                    nc.tensor.matmul(
                        pf, resT[:, t * 128 : (t + 1) * 128], wo, start=True, stop=True
                    )
                    ot = sp.tile([128, 128], f32, tag="out_sb")
                    nc.any.tensor_add(out=ot, in0=pf, in1=bias)
                    nc.sync.dma_start(y_sh[s, t * 128 : (t + 1) * 128, :], ot)
    nc.finalize()
    return nc


def _consts():
    mblk = np.zeros((128, 128), dtype=np.float32)
    msel = np.zeros((128, H), dtype=np.float32)
    for h in range(H):
        mblk[h * HD : (h + 1) * HD, h * HD : (h + 1) * HD] = 1.0
        msel[h * HD : (h + 1) * HD, h] = 1.0
    return np.eye(128, dtype=np.float32), mblk, msel


def kernel(x, W_qkv, W_out, b_out):
    if "nc" not in _cached:
        _cached["nc"] = _build()
    nc = _cached["nc"]
    iden, mblk, msel = _consts()
    xf = np.ascontiguousarray(x.reshape(B * T, N, D), dtype=np.float32)
    in_maps = []
    for c in range(NCORES):
        in_maps.append(
            {
                "x_sh": np.ascontiguousarray(xf[c * SLICES : (c + 1) * SLICES]),
                "w_qkv": np.ascontiguousarray(W_qkv, dtype=np.float32),
                "w_out": np.ascontiguousarray(W_out, dtype=np.float32),
                "b_out": np.ascontiguousarray(b_out, dtype=np.float32),
                "iden": iden,
                "mblk": mblk,
                "msel": msel,
            }
        )
    res = run_bass_kernel_spmd(nc, in_maps, core_ids=list(range(NCORES)))
    _cached["last"] = res
    out = np.concatenate([r["y_sh"] for r in res.results], axis=0)
    return out.reshape(B, T, N, D)

